# revision 4
# baseline (speedup 1.0000x reference)
"""Trainium2 Bass kernel: batched multi-head attention with per-frame
conditioning K/V token (nn_Attention dense_transformer problem).

Strategy: data-parallel over the 16 (b*n) frames -> 2 frames per NeuronCore,
no collectives. Per core, a fused kernel pipelined so the scalar engine's exp
stream (the softmax) never waits:

 - sim computed transposed (keys on partitions) so the PV matmul contracts
   over keys directly and softmax denominators come from a ones-column in
   the 65-wide PV stationary.
 - Both heads of a pair share one exp instruction: the QK row-tile pair
   writes sA/sB into adjacent PSUM banks of one [128, 2, 512] tile and a
   single activation covers 1024 elements/lane (halves ACT overhead).
 - The conditioning token is a 1-key "mini chunk" (no 127 dummy keys):
   QK writes single sim rows at partitions 0/32 of one bank, exp covers
   [33, 512], and PV uses contraction-1 stationaries. No big memsets.
 - PSUM budget: 4 banks sim (2-deep pipeline) + 2 banks PV accumulators +
   2 rotating work banks (projection chains / cond sim / denom broadcast).
 - QKV/out projections are emitted as background chains in fixed slots
   inside the attention sections so the PE fills its slack while ACT runs,
   and each section's sims are ready ahead of the exp stream.
"""

import numpy as np
import ml_dtypes

import concourse.bacc as bacc
import concourse.tile as tile
from concourse import mybir
from concourse.bass_utils import run_bass_kernel_spmd

BF16 = mybir.dt.bfloat16
F32 = mybir.dt.float32

HEADS = 8
DH = 64
D = 512
HID = 512
SCALE = DH ** -0.5
N_CORES = 8
NDC = D // 128  # 4 contraction chunks of 128


def build_attention_nc(T=1024, loop_n=1):
    NI = 512                # i-tile width (matmul moving free dim)
    NIH = T // NI           # i-tiles (token halves) per frame
    NTC = T // 128          # token chunks (key chunks / v chunks / out chunks)
    KT_PAD = 1032           # 1025 keys padded so (a, f) strides stay 16B-aligned

    nc = bacc.Bacc("TRN2", target_bir_lowering=False)
    x_d = nc.declare_dram_parameter("xT", [128, NDC, 2, T], BF16, isOutput=False)
    w_d = nc.declare_dram_parameter("Wqkv", [128, NDC, 3 * HID], BF16, isOutput=False)
    wk_d = nc.declare_dram_parameter("Wk", [128, NDC, HID], BF16, isOutput=False)
    wv_d = nc.declare_dram_parameter("Wv", [128, NDC, HID], BF16, isOutput=False)
    wo_d = nc.declare_dram_parameter("Wout", [128, NDC, D], BF16, isOutput=False)
    lab_d = nc.declare_dram_parameter("labT", [128, NDC, 2, 8], BF16, isOutput=False)
    f_d = nc.declare_dram_parameter("F", [33, 128], BF16, isOutput=False)
    out_d = nc.declare_dram_parameter("out", [2, T, D], BF16, isOutput=True)

    EXP = mybir.ActivationFunctionType.Exp

    with tile.TileContext(nc) as tc:
        with (
            tc.tile_pool(name="persist", bufs=1) as pp,
            tc.tile_pool(name="work", bufs=2) as wp,
            tc.tile_pool(name="psum", bufs=2, space="PSUM") as psp,
        ):
            def emit_body():
                # ---- persistent SBUF tiles ----
                xT = pp.tile([128, NDC, 2, T], BF16, tag="xT")
                wq = pp.tile([128, NDC, 3 * HID], BF16, tag="wq")
                wk = pp.tile([128, NDC, HID], BF16, tag="wk")
                wv = pp.tile([128, NDC, HID], BF16, tag="wv")
                wo = pp.tile([128, NDC, D], BF16, tag="wo")
                lab = pp.tile([128, NDC, 2, 8], BF16, tag="lab")
                qT = pp.tile([128, NDC, 2, T], BF16, tag="qT")
                kT = pp.tile([128, NDC, 2, KT_PAD], BF16, tag="kT")
                # v: real token chunks only; 72*2B=144B per-head stride (16B req)
                vv = pp.tile([128, 2, NTC, HEADS, 72], BF16, tag="vv")
                # cond-token v at partitions 0 (even heads) and 32 (odd heads)
                vcond = pp.tile([33, 2, HEADS, 72], BF16, tag="vcond")
                attn = pp.tile([128, NDC, 2, T], BF16, tag="attn")
                fmat = pp.tile([33, 128], BF16, tag="fmat")
                # 1/denom rows: 0 (h even) and 32 (h odd); rows 1-31 stay 1.0
                rg = pp.tile([33, NI], BF16, tag="rg")

                # ---- input DMAs, ordered for earliest compute start ----
                for dc in range(NDC):
                    nc.sync.dma_start(xT[:, dc, 0], x_d[:, dc, 0])
                nc.sync.dma_start(wq[:, :, 512:640], w_d[:, :, 512:640])      # k pair0
                nc.sync.dma_start(wq[:, :, 0:128], w_d[:, :, 0:128])          # q pair0
                nc.sync.dma_start(wq[:, :, 1024:1536], w_d[:, :, 1024:1536])  # v cols
                nc.sync.dma_start(wk[:], wk_d[:])
                nc.sync.dma_start(lab[:], lab_d[:])
                nc.sync.dma_start(wv[:], wv_d[:])
                nc.sync.dma_start(fmat[:], f_d[:])
                for p in range(1, 4):
                    nc.sync.dma_start(
                        wq[:, :, 512 + p * 128:512 + (p + 1) * 128],
                        w_d[:, :, 512 + p * 128:512 + (p + 1) * 128])
                    nc.sync.dma_start(
                        wq[:, :, p * 128:(p + 1) * 128],
                        w_d[:, :, p * 128:(p + 1) * 128])
                for dc in range(NDC):
                    nc.sync.dma_start(xT[:, dc, 1], x_d[:, dc, 1])
                nc.sync.dma_start(wo[:], wo_d[:])

                # constants (tiny)
                nc.vector.memset(rg[:], 1.0)
                nc.vector.memset(vv[:, :, :, :, DH:DH + 1], 1.0)
                nc.vector.memset(vcond[0:1, :, :, DH:DH + 1], 1.0)
                nc.vector.memset(vcond[32:33, :, :, DH:DH + 1], 1.0)

                # ---- projection chains (each ~1us of PE + one DVE copy) ----
                def emit_qk_proj(f, cc, ih):
                    """q (cc 0-3) or k (cc 4-7) projection for one token half."""
                    ps = psp.tile([128, NI], F32, tag="work")
                    isl = slice(ih * NI, (ih + 1) * NI)
                    for dc in range(NDC):
                        nc.tensor.matmul(
                            ps[:],
                            wq[:, dc, cc * 128:(cc + 1) * 128],
                            xT[:, dc, f, isl],
                            start=(dc == 0), stop=(dc == NDC - 1),
                        )
                    if cc < 4:
                        nc.vector.tensor_copy(qT[:, cc, f, isl], ps[:])
                    else:
                        nc.vector.tensor_copy(kT[:, cc - 4, f, isl], ps[:])

                def emit_v(f, tc_i):
                    ps = psp.tile([128, HID], F32, tag="work")
                    for dc in range(NDC):
                        nc.tensor.matmul(
                            ps[:],
                            xT[:, dc, f, tc_i * 128:(tc_i + 1) * 128],
                            wq[:, dc, 2 * HID:3 * HID],
                            start=(dc == 0), stop=(dc == NDC - 1),
                        )
                    nc.vector.tensor_copy(vv[:, f, tc_i, :, 0:DH], ps[:])

                def emit_ek():
                    """cond-token keys for both frames -> kT col T (=1024)."""
                    for cc in range(NDC):
                        ps = psp.tile([128, NI], F32, tag="work")
                        for dc in range(NDC):
                            nc.tensor.matmul(
                                ps[:, 0:2],
                                wk[:, dc, cc * 128:(cc + 1) * 128],
                                lab[:, dc, :, 0:1],
                                start=(dc == 0), stop=(dc == NDC - 1),
                            )
                        for f in range(2):
                            nc.vector.tensor_copy(kT[:, cc, f, T:T + 1], ps[:, f:f + 1])

                def emit_ev(f):
                    """cond-token v for frame f at partitions 0 and 32."""
                    for base in (0, 32):
                        ps = psp.tile([128, HID], F32, tag="work")
                        for dc in range(NDC):
                            nc.tensor.matmul(
                                ps[base:base + 1, :],
                                lab[:, dc, f, 0:1],
                                wv[:, dc, :],
                                start=(dc == 0), stop=(dc == NDC - 1),
                            )
                        nc.vector.tensor_copy(
                            vcond[base:base + 1, f, :, 0:DH], ps[base:base + 1, :])

                def emit_outproj(f, tc_i):
                    ps = psp.tile([128, D], F32, tag="work")
                    for a in range(NDC):
                        nc.tensor.matmul(
                            ps[:],
                            attn[:, a, f, tc_i * 128:(tc_i + 1) * 128],
                            wo[:, a, :],
                            start=(a == 0), stop=(a == NDC - 1),
                        )
                    ot = wp.tile([128, D], BF16, tag="oout")
                    nc.vector.tensor_copy(ot[:], ps[:])
                    nc.sync.dma_start(out_d[f, tc_i * 128:(tc_i + 1) * 128, :], ot[:])

                # ---- one attention section: pair a, token half ih, frame f ----
                def emit_section(f, a, ih, thunks):
                    """QK -> exp -> PV pipeline over 8 full key chunks plus the
                    1-key cond mini-chunk; `thunks` are background projection
                    chains slotted between chunks to fill PE slack."""
                    isl = slice(ih * NI, (ih + 1) * NI)
                    pvA = psp.tile([65, NI], F32, tag="pv")
                    pvB = psp.tile([65, NI], F32, tag="pv")
                    ti = 0

                    def pop_thunk():
                        nonlocal ti
                        if ti < len(thunks):
                            thunks[ti]()
                            ti += 1

                    sims = []
                    Ps = []

                    def emit_qk(jc):
                        simt = psp.tile([128, 2, NI], F32, tag="sim")
                        jsl = slice(jc * 128, (jc + 1) * 128)
                        nc.tensor.matmul(
                            simt[:, 0, :], kT[0:64, a, f, jsl], qT[0:64, a, f, isl],
                            start=True, stop=True, tile_position=(0, 0),
                        )
                        nc.tensor.matmul(
                            simt[:, 1, :], kT[64:128, a, f, jsl], qT[64:128, a, f, isl],
                            start=True, stop=True, tile_position=(64, 0),
                        )
                        sims.append(simt)

                    def emit_exp(jc):
                        P = wp.tile([128, 2, NI], BF16, tag="P", bufs=3)
                        nc.scalar.activation(P[:], sims[jc][:], EXP, scale=SCALE)
                        Ps.append(P)

                    def emit_pv(jc):
                        P = Ps[jc]
                        nc.tensor.matmul(
                            pvA[:], vv[:, f, jc, 2 * a, 0:65], P[:, 0, :],
                            start=(jc == 0), stop=False,
                        )
                        nc.tensor.matmul(
                            pvB[:], vv[:, f, jc, 2 * a + 1, 0:65], P[:, 1, :],
                            start=(jc == 0), stop=False,
                        )

                    emit_qk(0)
                    emit_exp(0)
                    emit_qk(1)
                    emit_exp(1)
                    pop_thunk()
                    for jc in range(2, NTC):
                        emit_qk(jc)
                        emit_exp(jc)
                        emit_pv(jc - 2)
                        pop_thunk()
                    # cond mini-chunk: sim rows at partitions 0 (even head) and
                    # 32 (odd head) of one work bank; exp covers [33, NI].
                    sim8 = psp.tile([33, NI], F32, tag="work")
                    nc.tensor.matmul(
                        sim8[0:1, :], kT[0:64, a, f, T:T + 1], qT[0:64, a, f, isl],
                        start=True, stop=True, tile_position=(0, 0),
                    )
                    nc.tensor.matmul(
                        sim8[32:33, :], kT[64:128, a, f, T:T + 1], qT[64:128, a, f, isl],
                        start=True, stop=True, tile_position=(64, 32),
                    )
                    P8 = wp.tile([33, NI], BF16, tag="P8")
                    nc.scalar.activation(P8[:], sim8[:], EXP, scale=SCALE)
                    emit_pv(NTC - 2)
                    pop_thunk()
                    emit_pv(NTC - 1)
                    nc.tensor.matmul(
                        pvA[:], vcond[0:1, f, 2 * a, 0:65], P8[0:1, :],
                        start=False, stop=True,
                    )
                    nc.tensor.matmul(
                        pvB[:], vcond[32:33, f, 2 * a + 1, 0:65], P8[32:33, :],
                        start=False, stop=True,
                    )
                    while ti < len(thunks):
                        pop_thunk()
                    # softmax denominators (row 64 of pvA/pvB) -> reciprocal ->
                    # broadcast across partitions via fmat matmul -> normalize
                    with nc.allow_low_precision("softmax denom reciprocal in bf16"):
                        nc.vector.reciprocal(rg[0:1, :], pvA[64:65, :])
                        nc.vector.reciprocal(rg[32:33, :], pvB[64:65, :])
                    bc = psp.tile([128, NI], F32, tag="work")
                    nc.tensor.matmul(bc[:], fmat[:], rg[:], start=True, stop=True)
                    rbc = wp.tile([128, NI], BF16, tag="rbc")
                    nc.vector.tensor_copy(rbc[:], bc[:])
                    nc.vector.tensor_mul(attn[0:64, a, f, isl], pvA[0:64, :], rbc[0:64, :])
                    nc.vector.tensor_mul(attn[64:128, a, f, isl], pvB[0:64, :], rbc[64:128, :])

                # ---- schedule ----
                K = lambda f, p, ih: (lambda: emit_qk_proj(f, 4 + p, ih))
                Q = lambda f, p, ih: (lambda: emit_qk_proj(f, p, ih))
                V = lambda f, t: (lambda: emit_v(f, t))
                EV = lambda f: (lambda: emit_ev(f))
                OP = lambda f, t: (lambda: emit_outproj(f, t))

                # preamble: minimum work before section (0,0,0) can stream
                emit_ek()
                emit_qk_proj(0, 4, 0)   # k(f0,p0,ih0)
                emit_qk_proj(0, 4, 1)   # k(f0,p0,ih1)
                emit_qk_proj(0, 0, 0)   # q(f0,p0,ih0)
                for t in range(4):
                    emit_v(0, t)
                emit_ev(0)

                sched = {
                    (0, 0, 0): [V(0, 4), V(0, 5), V(0, 6), V(0, 7), K(0, 1, 0), K(0, 1, 1), Q(0, 1, 0)],
                    (0, 0, 1): [K(0, 2, 0), K(0, 2, 1), Q(0, 2, 0)],
                    (0, 0, 2): [K(0, 3, 0), K(0, 3, 1), Q(0, 3, 0)],
                    (0, 0, 3): [Q(0, 0, 1)],
                    (0, 1, 0): [Q(0, 1, 1), V(1, 0), V(1, 1)],
                    (0, 1, 1): [Q(0, 2, 1), V(1, 2), V(1, 3)],
                    (0, 1, 2): [Q(0, 3, 1), V(1, 4), V(1, 5)],
                    (0, 1, 3): [V(1, 6), V(1, 7), EV(1), K(1, 0, 0), K(1, 0, 1), Q(1, 0, 0)],
                    (1, 0, 0): [K(1, 1, 0), K(1, 1, 1), Q(1, 1, 0), OP(0, 0)],
                    (1, 0, 1): [K(1, 2, 0), K(1, 2, 1), Q(1, 2, 0), OP(0, 1)],
                    (1, 0, 2): [K(1, 3, 0), K(1, 3, 1), Q(1, 3, 0), OP(0, 2)],
                    (1, 0, 3): [Q(1, 0, 1), OP(0, 3)],
                    (1, 1, 0): [Q(1, 1, 1), OP(1, 0), OP(1, 1)],
                    (1, 1, 1): [Q(1, 2, 1), OP(1, 2), OP(1, 3)],
                    (1, 1, 2): [Q(1, 3, 1), OP(0, 4), OP(0, 5)],
                    (1, 1, 3): [OP(0, 6), OP(0, 7)],
                }
                for f in range(2):
                    for ih in range(NIH):
                        for a in range(NDC):
                            emit_section(f, a, ih, sched.get((f, ih, a), []))
                for t in range(4, 8):
                    emit_outproj(1, t)

            if loop_n > 1:
                with tc.For_i(0, loop_n, 1):
                    emit_body()
            else:
                emit_body()

    nc.finalize()
    return nc


_NC_CACHE = {}


def _get_nc(T):
    if T not in _NC_CACHE:
        _NC_CACHE[T] = build_attention_nc(T)
    return _NC_CACHE[T]


def make_in_maps(x, label_emb_mm, Wqkv, Wk, Wv, Wout):
    """Host-side sharding + layout prep (transpose to feature-major, bf16)."""
    bf = ml_dtypes.bfloat16
    BN, T, d = x.shape
    assert (BN, d) == (16, D)
    # x[fr, t, dc*128+p] -> xB[fr, p, dc, t]
    xB = np.ascontiguousarray(
        np.asarray(x).reshape(16, T, NDC, 128).transpose(0, 3, 2, 1)
    ).astype(bf)
    wq = np.ascontiguousarray(np.asarray(Wqkv).reshape(NDC, 128, 3 * HID).transpose(1, 0, 2)).astype(bf)
    wkh = np.ascontiguousarray(np.asarray(Wk).reshape(NDC, 128, HID).transpose(1, 0, 2)).astype(bf)
    wvh = np.ascontiguousarray(np.asarray(Wv).reshape(NDC, 128, HID).transpose(1, 0, 2)).astype(bf)
    woh = np.ascontiguousarray(np.asarray(Wout).reshape(NDC, 128, D).transpose(1, 0, 2)).astype(bf)
    labB = np.asarray(label_emb_mm).reshape(16, NDC, 128)  # [fr, dc, p]
    F = np.zeros((33, 128), dtype=bf)
    F[0, 0:64] = 1.0
    F[32, 64:128] = 1.0
    in_maps = []
    for c in range(N_CORES):
        xTc = np.ascontiguousarray(xB[2 * c:2 * c + 2].transpose(1, 2, 0, 3))  # (128,4,2,T)
        labc2 = np.ascontiguousarray(labB[2 * c:2 * c + 2].transpose(2, 1, 0)).astype(bf)  # (128,4,2)
        labc = np.zeros((128, NDC, 2, 8), dtype=bf)  # padded so f-stride is 16B
        labc[:, :, :, 0] = labc2
        in_maps.append({
            "xT": xTc, "Wqkv": wq, "Wk": wkh, "Wv": wvh, "Wout": woh, "labT": labc,
            "F": F,
        })
    return in_maps


def kernel(x, label_emb_mm, Wqkv, Wk, Wv, Wout, b):
    x = np.asarray(x)
    T = x.shape[1]
    nc = _get_nc(T)
    in_maps = make_in_maps(x, label_emb_mm, Wqkv, Wk, Wv, Wout)
    res = run_bass_kernel_spmd(nc, in_maps, core_ids=list(range(N_CORES)))
    out = np.concatenate([res.results[c]["out"] for c in range(N_CORES)], axis=0)
    return np.ascontiguousarray(out.reshape(16, T, D)).astype(np.float32)


# revision 7
# speedup vs baseline: 1.3997x; 1.3997x over previous
"""Trainium2 Bass kernel: batched multi-head attention with per-frame
conditioning K/V token (nn_Attention dense_transformer problem).

Strategy: data-parallel over the 16 (b*n) frames -> 2 frames per NeuronCore,
no collectives. Per core, a fused kernel pipelined so the scalar engine's exp
stream (the softmax) rarely waits:

 - sim computed transposed (keys on partitions) so the PV matmul contracts
   over keys directly and softmax denominators come from a ones-column in
   the 65-wide PV stationary.
 - Both heads of a pair share one exp instruction: the QK row-tile pair
   writes sA/sB into adjacent PSUM banks of one [128, 2, 512] tile and a
   single activation covers 1024 elements/lane (halves ACT overhead).
 - The conditioning token is a 1-key mini-chunk (no 127 dummy keys):
   QK writes single sim rows at partitions 0/32 of one bank, exp covers
   [33, 512], and PV uses contraction-1 stationaries. No big memsets.
 - PSUM: 4 banks sim (2-deep QK->exp->PV pipeline) + 2 banks PV
   accumulators + 2 rotating work banks (proj chains / cond sim / bcast).
 - QKV/out projections run as background chains in fixed thunk slots
   inside the attention sections so the PE fills its slack while ACT runs.
 - bf16 output DMA (halves output traffic); host casts back to f32.
"""

import numpy as np
import ml_dtypes

import concourse.bacc as bacc
import concourse.tile as tile
from concourse import mybir
from concourse.bass_utils import run_bass_kernel_spmd

BF16 = mybir.dt.bfloat16
F32 = mybir.dt.float32

HEADS = 8
DH = 64
D = 512
HID = 512
SCALE = DH ** -0.5
N_CORES = 8
NDC = D // 128


def build_attention_nc(T=1024, loop_n=1):
    NI = 512
    NIH = T // NI
    NTC = T // 128
    KT_PAD = 1032

    nc = bacc.Bacc("TRN2", target_bir_lowering=False)
    x_d = nc.declare_dram_parameter("xT", [128, NDC, 2, T], BF16, isOutput=False)
    w_d = nc.declare_dram_parameter("Wqkv", [128, NDC, 3 * HID], BF16, isOutput=False)
    wk_d = nc.declare_dram_parameter("Wk", [128, NDC, HID], BF16, isOutput=False)
    wv_d = nc.declare_dram_parameter("Wv", [128, NDC, HID], BF16, isOutput=False)
    wo_d = nc.declare_dram_parameter("Wout", [128, NDC, D], BF16, isOutput=False)
    lab_d = nc.declare_dram_parameter("labT", [128, NDC, 2, 8], BF16, isOutput=False)
    f_d = nc.declare_dram_parameter("F", [33, 128], BF16, isOutput=False)
    out_d = nc.declare_dram_parameter("out", [2, T, D], BF16, isOutput=True)

    EXP = mybir.ActivationFunctionType.Exp

    with tile.TileContext(nc) as tc:
        with (
            tc.tile_pool(name="persist", bufs=1) as pp,
            tc.tile_pool(name="work", bufs=2) as wp,
            tc.tile_pool(name="psum", bufs=2, space="PSUM") as psp,
        ):
            def emit_body():
                xT = pp.tile([128, NDC, 2, T], BF16, tag="xT")
                wq = pp.tile([128, NDC, 3 * HID], BF16, tag="wq")
                wk = pp.tile([128, NDC, HID], BF16, tag="wk")
                wv = pp.tile([128, NDC, HID], BF16, tag="wv")
                wo = pp.tile([128, NDC, D], BF16, tag="wo")
                lab = pp.tile([128, NDC, 2, 8], BF16, tag="lab")
                qT = pp.tile([128, NDC, 2, T], BF16, tag="qT")
                kT = pp.tile([128, NDC, 2, KT_PAD], BF16, tag="kT")
                vv = pp.tile([128, 2, NTC, HEADS, 72], BF16, tag="vv")
                vcond = pp.tile([33, 2, HEADS, 72], BF16, tag="vcond")
                attn = pp.tile([128, NDC, 2, T], BF16, tag="attn")
                fmat = pp.tile([33, 128], BF16, tag="fmat")
                rg = pp.tile([33, NI], BF16, tag="rg")

                for dc in range(NDC):
                    nc.sync.dma_start(xT[:, dc, 0], x_d[:, dc, 0])
                nc.sync.dma_start(wq[:, :, 512:640], w_d[:, :, 512:640])
                nc.sync.dma_start(wq[:, :, 0:128], w_d[:, :, 0:128])
                nc.sync.dma_start(wq[:, :, 1024:1536], w_d[:, :, 1024:1536])
                nc.sync.dma_start(wk[:], wk_d[:])
                nc.sync.dma_start(lab[:], lab_d[:])
                nc.sync.dma_start(wv[:], wv_d[:])
                nc.sync.dma_start(fmat[:], f_d[:])
                for p in range(1, 4):
                    nc.sync.dma_start(
                        wq[:, :, 512 + p * 128:512 + (p + 1) * 128],
                        w_d[:, :, 512 + p * 128:512 + (p + 1) * 128])
                    nc.sync.dma_start(
                        wq[:, :, p * 128:(p + 1) * 128],
                        w_d[:, :, p * 128:(p + 1) * 128])
                for dc in range(NDC):
                    nc.sync.dma_start(xT[:, dc, 1], x_d[:, dc, 1])
                nc.sync.dma_start(wo[:], wo_d[:])

                nc.vector.memset(rg[:], 1.0)
                nc.vector.memset(vv[:, :, :, :, DH:DH + 1], 1.0)
                nc.vector.memset(vcond[0:1, :, :, DH:DH + 1], 1.0)
                nc.vector.memset(vcond[32:33, :, :, DH:DH + 1], 1.0)

                def emit_qk_proj(f, cc, ih):
                    ps = psp.tile([128, NI], F32, tag="work")
                    isl = slice(ih * NI, (ih + 1) * NI)
                    for dc in range(NDC):
                        nc.tensor.matmul(
                            ps[:],
                            wq[:, dc, cc * 128:(cc + 1) * 128],
                            xT[:, dc, f, isl],
                            start=(dc == 0), stop=(dc == NDC - 1),
                        )
                    if cc < 4:
                        nc.vector.tensor_copy(qT[:, cc, f, isl], ps[:])
                    else:
                        nc.vector.tensor_copy(kT[:, cc - 4, f, isl], ps[:])

                def emit_v(f, tc_i):
                    ps = psp.tile([128, HID], F32, tag="work")
                    for dc in range(NDC):
                        nc.tensor.matmul(
                            ps[:],
                            xT[:, dc, f, tc_i * 128:(tc_i + 1) * 128],
                            wq[:, dc, 2 * HID:3 * HID],
                            start=(dc == 0), stop=(dc == NDC - 1),
                        )
                    nc.vector.tensor_copy(vv[:, f, tc_i, :, 0:DH], ps[:])

                def emit_ek():
                    for cc in range(NDC):
                        ps = psp.tile([128, NI], F32, tag="work")
                        for dc in range(NDC):
                            nc.tensor.matmul(
                                ps[:, 0:2],
                                wk[:, dc, cc * 128:(cc + 1) * 128],
                                lab[:, dc, :, 0:1],
                                start=(dc == 0), stop=(dc == NDC - 1),
                            )
                        for f in range(2):
                            nc.vector.tensor_copy(kT[:, cc, f, T:T + 1], ps[:, f:f + 1])

                def emit_ev(f):
                    for base in (0, 32):
                        ps = psp.tile([128, HID], F32, tag="work")
                        for dc in range(NDC):
                            nc.tensor.matmul(
                                ps[base:base + 1, :],
                                lab[:, dc, f, 0:1],
                                wv[:, dc, :],
                                start=(dc == 0), stop=(dc == NDC - 1),
                            )
                        nc.vector.tensor_copy(
                            vcond[base:base + 1, f, :, 0:DH], ps[base:base + 1, :])

                def emit_outproj(f, tc_i):
                    ps = psp.tile([128, D], F32, tag="work")
                    for a in range(NDC):
                        nc.tensor.matmul(
                            ps[:],
                            attn[:, a, f, tc_i * 128:(tc_i + 1) * 128],
                            wo[:, a, :],
                            start=(a == 0), stop=(a == NDC - 1),
                        )
                    ot = wp.tile([128, D], BF16, tag="oout")
                    nc.vector.tensor_copy(ot[:], ps[:])
                    nc.sync.dma_start(out_d[f, tc_i * 128:(tc_i + 1) * 128, :], ot[:])

                def emit_section(f, a, ih, thunks):
                    isl = slice(ih * NI, (ih + 1) * NI)
                    pvA = psp.tile([65, NI], F32, tag="pv")
                    pvB = psp.tile([65, NI], F32, tag="pv")
                    ti = [0]

                    def pop_thunk():
                        if ti[0] < len(thunks):
                            thunks[ti[0]]()
                            ti[0] += 1

                    sims = []
                    Ps = []

                    def emit_qk(jc):
                        simt = psp.tile([128, 2, NI], F32, tag="sim")
                        jsl = slice(jc * 128, (jc + 1) * 128)
                        nc.tensor.matmul(
                            simt[:, 0, :], kT[0:64, a, f, jsl], qT[0:64, a, f, isl],
                            start=True, stop=True, tile_position=(0, 0),
                        )
                        nc.tensor.matmul(
                            simt[:, 1, :], kT[64:128, a, f, jsl], qT[64:128, a, f, isl],
                            start=True, stop=True, tile_position=(64, 0),
                        )
                        sims.append(simt)

                    def emit_exp(jc):
                        P = wp.tile([128, 2, NI], BF16, tag="P", bufs=3)
                        nc.scalar.activation(P[:], sims[jc][:], EXP, scale=SCALE)
                        Ps.append(P)

                    def emit_pv(jc):
                        P = Ps[jc]
                        nc.tensor.matmul(
                            pvA[:], vv[:, f, jc, 2 * a, 0:65], P[:, 0, :],
                            start=(jc == 0), stop=False,
                        )
                        nc.tensor.matmul(
                            pvB[:], vv[:, f, jc, 2 * a + 1, 0:65], P[:, 1, :],
                            start=(jc == 0), stop=False,
                        )

                    emit_qk(0)
                    emit_exp(0)
                    emit_qk(1)
                    emit_exp(1)
                    pop_thunk()
                    for jc in range(2, NTC):
                        emit_qk(jc)
                        emit_exp(jc)
                        emit_pv(jc - 2)
                        pop_thunk()
                    sim8 = psp.tile([33, NI], F32, tag="work")
                    nc.tensor.matmul(
                        sim8[0:1, :], kT[0:64, a, f, T:T + 1], qT[0:64, a, f, isl],
                        start=True, stop=True, tile_position=(0, 0),
                    )
                    nc.tensor.matmul(
                        sim8[32:33, :], kT[64:128, a, f, T:T + 1], qT[64:128, a, f, isl],
                        start=True, stop=True, tile_position=(64, 32),
                    )
                    P8 = wp.tile([33, NI], BF16, tag="P8")
                    nc.scalar.activation(P8[:], sim8[:], EXP, scale=SCALE)
                    emit_pv(NTC - 2)
                    pop_thunk()
                    emit_pv(NTC - 1)
                    nc.tensor.matmul(
                        pvA[:], vcond[0:1, f, 2 * a, 0:65], P8[0:1, :],
                        start=False, stop=True,
                    )
                    nc.tensor.matmul(
                        pvB[:], vcond[32:33, f, 2 * a + 1, 0:65], P8[32:33, :],
                        start=False, stop=True,
                    )
                    while ti[0] < len(thunks):
                        pop_thunk()
                    with nc.allow_low_precision("softmax denom reciprocal in bf16"):
                        nc.vector.reciprocal(rg[0:1, :], pvA[64:65, :])
                        nc.vector.reciprocal(rg[32:33, :], pvB[64:65, :])
                    bc = psp.tile([128, NI], F32, tag="work")
                    nc.tensor.matmul(bc[:], fmat[:], rg[:], start=True, stop=True)
                    rbc = wp.tile([128, NI], BF16, tag="rbc")
                    nc.vector.tensor_copy(rbc[:], bc[:])
                    nc.vector.tensor_mul(attn[0:64, a, f, isl], pvA[0:64, :], rbc[0:64, :])
                    nc.vector.tensor_mul(attn[64:128, a, f, isl], pvB[0:64, :], rbc[64:128, :])

                K = lambda f, p, ih: (lambda: emit_qk_proj(f, 4 + p, ih))
                Q = lambda f, p, ih: (lambda: emit_qk_proj(f, p, ih))
                V = lambda f, t: (lambda: emit_v(f, t))
                EV = lambda f: (lambda: emit_ev(f))
                OP = lambda f, t: (lambda: emit_outproj(f, t))

                emit_ek()
                emit_qk_proj(0, 4, 0)
                emit_qk_proj(0, 4, 1)
                emit_qk_proj(0, 0, 0)
                for t in range(4):
                    emit_v(0, t)
                emit_ev(0)

                sched = {
                    (0, 0, 0): [V(0, 4), V(0, 5), V(0, 6), V(0, 7), K(0, 1, 0), K(0, 1, 1), Q(0, 1, 0)],
                    (0, 0, 1): [K(0, 2, 0), K(0, 2, 1), Q(0, 2, 0)],
                    (0, 0, 2): [K(0, 3, 0), K(0, 3, 1), Q(0, 3, 0)],
                    (0, 0, 3): [Q(0, 0, 1)],
                    (0, 1, 0): [Q(0, 1, 1), V(1, 0), V(1, 1)],
                    (0, 1, 1): [Q(0, 2, 1), V(1, 2), V(1, 3)],
                    (0, 1, 2): [Q(0, 3, 1), V(1, 4), V(1, 5)],
                    (0, 1, 3): [V(1, 6), V(1, 7), EV(1), K(1, 0, 0), K(1, 0, 1), Q(1, 0, 0)],
                    (1, 0, 0): [K(1, 1, 0), K(1, 1, 1), Q(1, 1, 0), OP(0, 0)],
                    (1, 0, 1): [K(1, 2, 0), K(1, 2, 1), Q(1, 2, 0), OP(0, 1)],
                    (1, 0, 2): [K(1, 3, 0), K(1, 3, 1), Q(1, 3, 0), OP(0, 2)],
                    (1, 0, 3): [Q(1, 0, 1), OP(0, 3)],
                    (1, 1, 0): [Q(1, 1, 1), OP(1, 0), OP(1, 1)],
                    (1, 1, 1): [Q(1, 2, 1), OP(1, 2), OP(1, 3)],
                    (1, 1, 2): [Q(1, 3, 1), OP(0, 4), OP(0, 5)],
                    (1, 1, 3): [OP(0, 6), OP(0, 7)],
                }
                for f in range(2):
                    for ih in range(NIH):
                        for a in range(NDC):
                            emit_section(f, a, ih, sched.get((f, ih, a), []))
                for t in range(4, 8):
                    emit_outproj(1, t)

            if loop_n > 1:
                with tc.For_i(0, loop_n, 1):
                    emit_body()
            else:
                emit_body()

    nc.finalize()
    return nc


_NC_CACHE = {}


def _get_nc(T):
    if T not in _NC_CACHE:
        _NC_CACHE[T] = build_attention_nc(T)
    return _NC_CACHE[T]


def make_in_maps(x, label_emb_mm, Wqkv, Wk, Wv, Wout):
    bf = ml_dtypes.bfloat16
    BN, T, d = x.shape
    assert (BN, d) == (16, D)
    xB = np.ascontiguousarray(
        np.asarray(x).reshape(16, T, NDC, 128).transpose(0, 3, 2, 1)
    ).astype(bf)
    wq = np.ascontiguousarray(np.asarray(Wqkv).reshape(NDC, 128, 3 * HID).transpose(1, 0, 2)).astype(bf)
    wkh = np.ascontiguousarray(np.asarray(Wk).reshape(NDC, 128, HID).transpose(1, 0, 2)).astype(bf)
    wvh = np.ascontiguousarray(np.asarray(Wv).reshape(NDC, 128, HID).transpose(1, 0, 2)).astype(bf)
    woh = np.ascontiguousarray(np.asarray(Wout).reshape(NDC, 128, D).transpose(1, 0, 2)).astype(bf)
    labB = np.asarray(label_emb_mm).reshape(16, NDC, 128)
    F = np.zeros((33, 128), dtype=bf)
    F[0, 0:64] = 1.0
    F[32, 64:128] = 1.0
    in_maps = []
    for c in range(N_CORES):
        xTc = np.ascontiguousarray(xB[2 * c:2 * c + 2].transpose(1, 2, 0, 3))
        labc2 = np.ascontiguousarray(labB[2 * c:2 * c + 2].transpose(2, 1, 0)).astype(bf)
        labc = np.zeros((128, NDC, 2, 8), dtype=bf)
        labc[:, :, :, 0] = labc2
        in_maps.append({
            "xT": xTc, "Wqkv": wq, "Wk": wkh, "Wv": wvh, "Wout": woh, "labT": labc,
            "F": F,
        })
    return in_maps


def kernel(x, label_emb_mm, Wqkv, Wk, Wv, Wout, b):
    x = np.asarray(x)
    T = x.shape[1]
    nc = _get_nc(T)
    in_maps = make_in_maps(x, label_emb_mm, Wqkv, Wk, Wv, Wout)
    res = run_bass_kernel_spmd(nc, in_maps, core_ids=list(range(N_CORES)))
    out = np.concatenate([res.results[c]["out"] for c in range(N_CORES)], axis=0)
    return np.ascontiguousarray(out.reshape(16, T, D)).astype(np.float32)
